# revision 53
# baseline (speedup 1.0000x reference)
"""AnyLoc VLAD (vq_codebook) Trainium2 kernel, 8-core data parallel. v4.

Reference computation (per image, N=1024 patches, K=64 clusters, D=1536):
  descs_n = l2norm(query_descs)                 # row-normalize descriptors
  labels  = argmax_k(descs_n . l2norm(centers)) # hard assignment
  sum_d_k = sum_{n: label=k} descs_n            # per-cluster sum
  un_vlad = sum_d_k - count_k * centers_k
  vlad    = l2norm_rows(un_vlad); flatten; l2norm

Sharding: data-parallel over the batch axis, 4 images per NeuronCore; each
core holds the whole (tiny) codebook; host concatenates the per-core
outputs (no collectives needed).

Final structure (trace-driven: the two HWDGE rings sustain ~420 GB/s
aggregate but arbitrate lumpily at transfer granularity, so ring queues
are kept shallow; PE warm-equivalent busy ~28us; the HAM clock gate
halves PE speed after any ~3.4us idle window):

  - input DMAs roll with a 3-slot lookahead (ring queues stay ~3 deep,
    bounding worst-case arbitration hogs): sync ring = cnt2 + tsp slots,
    scalar ring = nat slots + merged centers|identity const, both packed
    host-side so every transfer is contiguous 3-6KB-per-partition rows.
    Slot 0's tsp is split in two so the first sims matmuls start ~2us
    earlier. tsp/nat pools hold all 8 slots (no recycling waits).
  - PE warm-up: a run of dummy matmuls on a memset tile keeps the PE HAM
    activity window busy through the framework preamble so the real sims
    matmuls run at 2.4 GHz from the start.
  - per-slot pipeline as v3: 6 DoubleRow fp8 sims matmuls (codebook
    stationary), ACT copy to bf16, 4 PE transposes back to patch-major,
    DVE row-max + is_ge one-hot; transposes deferred one slot, aggregation
    two slots so TensorE never waits mid-stream.
  - aggregation groups all matmuls sharing one stationary operand
    ([agg jj0..2, counts] per chunk-pair) so walrus can elide LDWEIGHTS.
  - finalize: the zero-row gate for the global norm comes from COUNTS
    (un_vlad row k is exactly 0 iff count_k == 0), so the global-norm
    branch runs as soon as counts stop, off the critical path. The last
    image's finalize is pipelined per 512-column chunk: each chunk's
    -64*counts*centers fold-in matmul is chased by its ACT Square+accum,
    so only the last chunk's square sits on the tail.
  - out DMAs ride the (otherwise idle) GPSIMD SWDGE queue so they never
    block input issues; the final image's outs use the sync HWDGE ring
    (lower first-byte latency, inputs long done).
  - PSUM budget exactly 8 banks: 2 sims + 2 transpose/warmup + 1 counts
    + 3 agg.

Toolchain workarounds: this walrus build accepts only one sync wait per
instruction, so Tile's tail drain is re-spread across per-engine drains
and a post-pass hoists surplus waits onto no-op carriers.
"""

import os
import sys

import numpy as np

for _p in ("/opt/trn_rl_repo", "/root/.axon_site/_ro/trn_rl_repo"):
    if os.path.isdir(_p) and _p not in sys.path:
        sys.path.insert(0, _p)

from contextlib import ExitStack

import ml_dtypes
import bass_rust
import concourse.bass as bass
import concourse.tile as tile
from concourse import bass_isa, library_config, mybir
from concourse.bass_utils import run_bass_kernel_spmd

B, N, K, D = 32, 1024, 64, 1536
NCORES = 8
IMGS = B // NCORES  # images per core
P = 128
NPAIR = 4   # patch chunk-pairs per image (N = NPAIR*256)
CP = 6      # feature chunk-pairs (D = CP*256)
JJ = D // 512  # agg column blocks
DH = D // 2    # finalize half split
WARM_MMS = 10  # PE warm-up matmuls (N=256 each, ~0.2us cold apiece)
BF16 = mybir.dt.bfloat16
FP8 = mybir.dt.float8e4
F32 = mybir.dt.float32
NP_BF16 = ml_dtypes.bfloat16
NP_FP8 = ml_dtypes.float8_e4m3
Alu = mybir.AluOpType
Act = mybir.ActivationFunctionType
DR = mybir.MatmulPerfMode.DoubleRow
EPS = 1e-12


def _patch_tile_drain():
    """This walrus build only accepts ONE sync wait per instruction; Tile's
    tail drain aggregates every outstanding semaphore wait onto a single
    Drain. Spread the waits across extra per-engine drains (all still
    before the end-of-kernel barrier, so semantics are unchanged)."""
    if getattr(tile.TileContext, "_vlad_drain_patched", False):
        return
    from concourse.vector_clock import ScopedClock

    def patched(self, tick_clock, wait_clock):
        nc = self.nc
        probe = nc.sync.drain()
        wait_clock.add_sem_waits(
            probe.ins, ScopedClock({None: tick_clock.global_clock})
        )
        si = probe.ins.sync_info
        waits = list(si.on_wait) if si is not None else []
        upds = list(si.on_update) if si is not None else []
        probe.ins.sync_info = bass_rust.SyncInfo(on_wait=waits[:1], on_update=upds)
        engines = [nc.scalar, nc.vector, nc.tensor, nc.gpsimd, nc.sync]
        for i, w in enumerate(waits[1:]):
            d = engines[i % len(engines)].drain()
            dsi = d.ins.sync_info
            du = list(dsi.on_update) if dsi is not None else []
            d.ins.sync_info = bass_rust.SyncInfo(on_wait=[w], on_update=du)
        nc.all_engine_barrier()
        popped = nc._tile_sem_poison_stack.pop()
        assert popped is self._sem_poison
        nc.clear_and_free_semaphores(list(self.sems.allocated().values()))

    tile.TileContext._drain_and_barrier = patched
    tile.TileContext._vlad_drain_patched = True


def _split_multi_waits(nc):
    """Walrus here accepts only one sync wait per instruction. Hoist surplus
    waits onto no-op carrier instructions inserted just before, on the same
    engine (safe: same engine executes in order, so all waits still complete
    before the original instruction issues)."""
    n_new = 0
    for _bbname, bassbb in list(nc.bb_map.items()):
        bb = bassbb.bb
        out = []
        changed = False
        for ins in bb.instructions:
            si = getattr(ins, "sync_info", None)
            waits = list(si.on_wait) if si is not None else []
            if len(waits) > 1:
                changed = True
                for w in waits[:-1]:
                    n_new += 1
                    nop = mybir.InstNoOp(
                        name=f"{ins.name}-wsplit{n_new}",
                        sync_info=mybir.SyncInfo(on_wait=[w], on_update=[]),
                        bass_nofuse=True,
                        engine=ins.engine,
                    )
                    nc.register_instruction(nop)
                    out.append(nop)
                ins.sync_info = bass_rust.SyncInfo(
                    on_wait=[waits[-1]], on_update=list(si.on_update)
                )
            out.append(ins)
        if changed:
            bb.instructions = out
    return n_new


def _slot_geom(npair):
    """Half-image slotting: S slots per image, pps chunk-pairs per slot."""
    S = 2 if npair % 2 == 0 and npair >= 2 else 1
    pps = npair // S
    nsl = pps * 2 * P  # patch columns per slot
    return S, pps, nsl


def build_nc(imgs=IMGS, npair=NPAIR):
    """Build the per-core Bass graph. `imgs`/`npair` shrinkable for sim."""
    _patch_tile_drain()
    S, pps, nsl = _slot_geom(npair)
    nch_s = 2 * pps  # 128-patch chunks per slot
    nslots = imgs * S

    nc = bass.Bass("TRN2", target_bir_lowering=False, debug=False)
    # natural pair tiles: row (slot, p) = 6KB [cp, q, d] flat, where
    # element (cp, q, d) = desc[chunk 2*(slot_pairbase+cp)+q, patch p, d]
    descsn_e = nc.dram_tensor("descsn", [imgs * S * P, pps * 2 * D], FP8,
                              kind="ExternalInput")
    # DoubleRow-packed transpose: row (slot, p) = 6KB [c, q, n] flat with
    # element (c, q, n) = desc[b, slot_n0 + n, 256c+128q+p]
    descst_e = nc.dram_tensor("descst", [imgs * S * P, CP * 2 * nsl], FP8,
                              kind="ExternalInput")
    # codebook, same DoubleRow packing, pre-packed host-side so the DMA is a
    # plain contiguous [128, 768B] read: row p = [c, q, k] flat with
    # element (c, q, k) = cnorm64[k, 256c+128q+p]
    cnt2_e = nc.dram_tensor("cnt2", [P, CP * 2 * K], FP8, kind="ExternalInput")
    # centers (bf16) with the KxK identity appended on the free axis
    cenid_e = nc.dram_tensor("cenid", [K, D + K], BF16, kind="ExternalInput")
    out_e = nc.dram_tensor("out", [imgs, K * D], BF16, kind="ExternalOutput")

    with tile.TileContext(nc) as tc:
        with ExitStack() as ctx:
            consts = ctx.enter_context(tc.tile_pool(name="consts", bufs=1))
            tspp = ctx.enter_context(tc.tile_pool(name="tspp", bufs=nslots))
            natp = ctx.enter_context(tc.tile_pool(name="natp", bufs=nslots))
            simsbp = ctx.enter_context(tc.tile_pool(name="simsbp", bufs=4))
            asnp = ctx.enter_context(tc.tile_pool(name="asnp", bufs=3))
            mxp = ctx.enter_context(tc.tile_pool(name="mxp", bufs=3))
            uvp = ctx.enter_context(tc.tile_pool(name="uvp", bufs=2))
            sqp = ctx.enter_context(tc.tile_pool(name="sqp", bufs=2))
            vfinp = ctx.enter_context(tc.tile_pool(name="vfinp", bufs=max(imgs, 2)))
            finp = ctx.enter_context(tc.tile_pool(name="finp", bufs=16))
            simsps = ctx.enter_context(
                tc.tile_pool(name="simsps", bufs=2, space="PSUM"))
            transps = ctx.enter_context(
                tc.tile_pool(name="transps", bufs=2, space="PSUM"))
            cntps = ctx.enter_context(
                tc.tile_pool(name="cntps", bufs=1, space="PSUM"))
            aggps = ctx.enter_context(
                tc.tile_pool(name="aggps", bufs=1, space="PSUM"))

            # ---- PE warm-up: dummy matmuls on a memset tile so the HAM
            # activity window is busy through the framework preamble and the
            # first real sims run at full clock. Output goes to a transpose-
            # pool psum bank that slot-0's transposes will overwrite later.
            wsrc = consts.tile([P, 2 * P], BF16)
            nc.vector.memset(wsrc, 0.03125)
            warm = transps.tile([P, nch_s, K], F32, tag="tr", name="warm")
            wview = warm.rearrange("p a b -> p (a b)")
            for _ in range(WARM_MMS):
                nc.tensor.matmul(
                    wview, lhsT=wsrc[:, 0:P], rhs=wsrc,
                    start=True, stop=True, skip_group_check=True,
                )

            def pe_fill(n):
                """Zero-dependency PE filler: standalone weight loads (no
                PSUM write, no sems) that execute the instant the PE queue
                reaches them. Padding the queue before sem-waiting matmuls
                keeps the HAM activity window busy so the clock gate stays
                at 8/8 through dependency gaps."""
                for _ in range(n):
                    nc.tensor.ldweights(wsrc[:, 0:P])

            # bridge from the warm-up burst to the first (DMA-gated) sims
            pe_fill(8)

            # ---- input DMA plan ----
            # The two HWDGE rings together sustain ~420 GB/s, but the
            # arbiter hogs at TRANSFER granularity: whatever is queued on
            # the favored ring runs before the other ring gets service,
            # and which ring wins is racy. Lumps are therefore bounded by
            # ring queue depth — so issues roll with only ONE slot of
            # lookahead (each ring ≤2 transfers deep, worst-case lump
            # ~2us) instead of deep prefetch. tsp rides sync, nat rides
            # scalar, and the deep pipeline stagger below absorbs the
            # residual arrival jitter.
            tsps = [None] * nslots
            nats = [None] * nslots

            def issue_tsp(t, eng):
                tsp = tspp.tile([P, CP, 2, nsl], FP8, tag="tsp",
                                name=f"tsp{t}")
                src = descst_e.ap()[t * P:(t + 1) * P, :].rearrange(
                    "p (c q n) -> p c q n", c=CP, q=2)
                if t == 0:
                    # split so the first sims matmuls start ~2us earlier
                    h = CP // 2
                    eng.dma_start(out=tsp[:, 0:h], in_=src[:, 0:h])
                    eng.dma_start(out=tsp[:, h:CP], in_=src[:, h:CP])
                else:
                    eng.dma_start(out=tsp, in_=src)
                tsps[t] = tsp

            def issue_nat(t, eng):
                nat = natp.tile([P, pps, 2, D], FP8, tag="nat",
                                name=f"nat{t}")
                eng.dma_start(
                    out=nat,
                    in_=descsn_e.ap()[t * P:(t + 1) * P, :]
                    .rearrange("p (c q d) -> p c q d", c=pps, q=2),
                )
                nats[t] = nat

            cnt_sb = consts.tile([P, CP, 2, K], FP8)
            nc.sync.dma_start(
                out=cnt_sb,
                in_=cnt2_e.ap().rearrange("p (c q k) -> p c q k", c=CP, q=2),
            )
            cenid_sb = consts.tile([K, D + K], BF16)
            nc.scalar.dma_start(out=cenid_sb, in_=cenid_e.ap())
            for t in range(min(3, nslots)):
                issue_tsp(t, nc.sync)
                if t < 2:
                    # nat lookahead is shallower: the scalar ring's early
                    # queue (cenid+nat0+nat1 ~1.8MB) bounds how long a
                    # worst-case arbitration hog can starve the sync ring's
                    # fill-critical tsp0
                    issue_nat(t, nc.scalar)
            cen_sb = cenid_sb[:, 0:D]
            ident_sb = cenid_sb[:, D:D + K]
            onesc = consts.tile([P, 2, 1], FP8)
            nc.vector.memset(onesc, 1.0)
            onesg = consts.tile([K, K], BF16)
            nc.vector.memset(onesg, 1.0)

            img_agg = {}

            def emit_agg(s, last_img=False):
                """Aggregate slot s's one-hot (deferred one slot). Matmuls
                sharing a stationary operand are grouped ([agg jj*, counts]
                per cp) so walrus can skip redundant LDWEIGHTS. For the
                final slot of the final image the agg runs jj-major with the
                -64*counts*centers fold-in matmul and its ACT Square chasing
                each chunk, so only the last chunk's square is on the tail."""
                b, sj = s["b"], s["sj"]
                if sj == 0:
                    img_agg[b] = dict(
                        agg=aggps.tile([K, JJ, 512], F32, tag="agg", name="agg"),
                        counts=cntps.tile([K, 1], F32, tag="cnt", name="counts"),
                    )
                ia = img_agg[b]
                final = last_img and sj == S - 1
                if final:
                    # counts first so the fold-in diag is ready per chunk
                    for cp in range(pps):
                        nc.tensor.matmul(
                            ia["counts"], lhsT=s["asn"][:, 2 * cp:2 * cp + 2, :],
                            rhs=onesc, start=(sj == 0 and cp == 0),
                            stop=(cp == pps - 1),
                            perf_mode=DR, skip_group_check=True,
                        )
                    diag = finp.tile([K, K], BF16, tag="diag")
                    nc.vector.tensor_scalar(
                        diag, ident_sb, scalar1=ia["counts"], scalar2=-64.0,
                        op0=Alu.mult, op1=Alu.mult)
                    sq = sqp.tile([K, D], FP8, tag="sq")
                    r2j = [finp.tile([K, 1], F32, tag=f"r2j{jj}",
                                     name=f"r2j{jj}")
                           for jj in range(JJ)]
                    for jj in range(JJ):
                        pe_fill(4)
                        for cp in range(pps):
                            nc.tensor.matmul(
                                ia["agg"][:, jj, :],
                                lhsT=s["asn"][:, 2 * cp:2 * cp + 2, :],
                                rhs=s["nat"][:, cp, :, jj * 512:(jj + 1) * 512],
                                start=(sj == 0 and cp == 0), stop=False,
                                perf_mode=DR, skip_group_check=True,
                            )
                        nc.tensor.matmul(
                            ia["agg"][:, jj, :], lhsT=diag,
                            rhs=cen_sb[:, jj * 512:(jj + 1) * 512],
                            start=False, stop=True, skip_group_check=True,
                        )
                        nc.scalar.activation(
                            sq[:, jj * 512:(jj + 1) * 512],
                            ia["agg"][:, jj, :], Act.Square, accum_out=r2j[jj])
                    s["r2j"] = r2j
                    return
                for cp in range(pps):
                    for jj in range(JJ):
                        nc.tensor.matmul(
                            ia["agg"][:, jj, :],
                            lhsT=s["asn"][:, 2 * cp:2 * cp + 2, :],
                            rhs=s["nat"][:, cp, :, jj * 512:(jj + 1) * 512],
                            start=(sj == 0 and cp == 0), stop=False,
                            perf_mode=DR, skip_group_check=True,
                        )
                    nc.tensor.matmul(
                        ia["counts"], lhsT=s["asn"][:, 2 * cp:2 * cp + 2, :],
                        rhs=onesc,
                        start=(sj == 0 and cp == 0),
                        stop=(sj == S - 1 and cp == pps - 1),
                        perf_mode=DR, skip_group_check=True,
                    )

            def emit_gnorm(ia):
                """Global-norm branch straight from counts: un_vlad row k is
                exactly 0 iff count_k == 0, so the nonzero-row gate (and the
                whole 1/sqrt(#nonzero) chain) runs as soon as counts stop,
                in parallel with the residual fold-in and row norms."""
                pe_fill(2)
                sgate = finp.tile([K, 1], BF16, tag="sgate")
                nc.vector.tensor_scalar(
                    sgate, ia["counts"], scalar1=1e30, scalar2=1.0,
                    op0=Alu.mult, op1=Alu.min,
                )
                g_ps = transps.tile([K, 1], F32, tag="tr", name="g_ps")
                nc.tensor.matmul(g_ps, lhsT=onesg, rhs=sgate,
                                 start=True, stop=True, skip_group_check=True)
                gr = finp.tile([K, 1], F32, tag="gr")
                nc.vector.reciprocal(gr, g_ps)
                ginvs = finp.tile([K, 1], F32, tag="ginvs")
                nc.scalar.sqrt(ginvs, gr)  # 1/sqrt(#nonzero rows)
                return ginvs

            def emit_fin1(b):
                """Image finalize part 1 (non-final images): un_vlad fold-in
                + row norm accumulation. un_vlad is copied to SBUF in
                ACT/DVE halves, releasing the agg banks early so the next
                image's aggregation never stalls on this finalize."""
                ia = img_agg.pop(b)
                gr = emit_gnorm(ia)
                diag = finp.tile([K, K], BF16, tag="diag")
                nc.vector.tensor_scalar(
                    diag, ident_sb, scalar1=ia["counts"], scalar2=-64.0,
                    op0=Alu.mult, op1=Alu.mult)
                for jj in range(JJ):
                    nc.tensor.matmul(
                        ia["agg"][:, jj, :], lhsT=diag,
                        rhs=cen_sb[:, jj * 512:(jj + 1) * 512],
                        start=False, stop=(jj == JJ - 1),
                        skip_group_check=True,
                    )
                aggflat = ia["agg"].rearrange("k a b -> k (a b)")
                uv = uvp.tile([K, D], F32, tag="uv")
                nc.scalar.mul(uv[:, 0:DH], aggflat[:, 0:DH], 1.0)
                nc.vector.tensor_scalar_mul(uv[:, DH:D], aggflat[:, DH:D], 1.0)
                sq = sqp.tile([K, D], FP8, tag="sq")
                r2a = finp.tile([K, 1], F32, tag="r2a")
                nc.scalar.activation(sq[:, 0:DH], uv[:, 0:DH], Act.Square,
                                     accum_out=r2a)
                r2b = finp.tile([K, 1], F32, tag="r2b")
                nc.vector.scalar_tensor_tensor(
                    out=sq[:, DH:D], in0=uv[:, DH:D], scalar=1.0,
                    in1=uv[:, DH:D], op0=Alu.mult, op1=Alu.mult,
                    accum_out=r2b,
                )
                # r2 = max(r2a, EPS^2) + r2b — the add and the zero-row
                # guard (reference: max(||row||, 1e-12)) fused in one op
                r2 = finp.tile([K, 1], F32, tag="r2")
                nc.vector.scalar_tensor_tensor(
                    out=r2, in0=r2a, scalar=EPS * EPS, in1=r2b,
                    op0=Alu.max, op1=Alu.add,
                )
                u = finp.tile([K, 1], F32, tag="u")
                nc.scalar.sqrt(u, r2)
                return dict(b=b, uv=uv, u=u, ginvs=gr)

            def emit_fin2(f):
                """Image finalize part 2: scales + output DMA (SWDGE queue,
                so input issues on sync/scalar are never blocked)."""
                invu = finp.tile([K, 1], F32, tag="invu")
                nc.vector.reciprocal(invu, f["u"])
                tot = finp.tile([K, 1], F32, tag="tot")
                nc.vector.tensor_mul(tot, invu, f["ginvs"])
                vfin = vfinp.tile([K, D], BF16, tag="vfin")
                out_kd = out_e.ap()[f["b"]].rearrange("(k d) -> k d", k=K)
                nc.scalar.mul(vfin[:, 0:DH], f["uv"][:, 0:DH], tot)
                nc.gpsimd.dma_start(out=out_kd[:, 0:DH], in_=vfin[:, 0:DH])
                nc.vector.tensor_scalar_mul(
                    vfin[:, DH:D], f["uv"][:, DH:D], tot)
                nc.gpsimd.dma_start(out=out_kd[:, DH:D], in_=vfin[:, DH:D])

            def emit_fin_final(s):
                """Final image finalize: squares already accumulated per
                chunk inside emit_agg; finish r2, scales, and out DMAs on
                the (long idle) sync ring."""
                b = s["b"]
                ia = img_agg.pop(b)
                gr = emit_gnorm(ia)
                r2j = s["r2j"]
                r2p = finp.tile([K, 1], F32, tag="r2p")
                nc.vector.tensor_tensor(r2p, r2j[0], r2j[1], op=Alu.add)
                r2 = finp.tile([K, 1], F32, tag="r2")
                nc.vector.tensor_tensor(r2, r2p, r2j[2], op=Alu.add)
                u = finp.tile([K, 1], F32, tag="u")
                nc.scalar.sqrt(u, r2)
                um = finp.tile([K, 1], F32, tag="um")
                nc.vector.tensor_scalar_max(um, u, EPS)
                invu = finp.tile([K, 1], F32, tag="invu")
                nc.vector.reciprocal(invu, um)
                tot = finp.tile([K, 1], F32, tag="tot")
                nc.vector.tensor_mul(tot, invu, gr)
                aggflat = ia["agg"].rearrange("k a b -> k (a b)")
                vfin = vfinp.tile([K, D], BF16, tag="vfin")
                out_kd = out_e.ap()[b].rearrange("(k d) -> k d", k=K)
                nc.scalar.mul(vfin[:, 0:DH], aggflat[:, 0:DH], tot)
                nc.sync.dma_start(out=out_kd[:, 0:DH], in_=vfin[:, 0:DH])
                nc.vector.tensor_scalar_mul(
                    vfin[:, DH:D], aggflat[:, DH:D], tot)
                nc.sync.dma_start(out=out_kd[:, DH:D], in_=vfin[:, DH:D])

            # The PE drops to a low p-state after any idle gap and needs
            # ~3us of continuous execution to reach full clock, so the slot
            # pipeline is staged so that EVERY tensor instruction's inputs
            # are ready before the engine reaches it: transposes run one
            # slot behind sims (their ACT copy finished last stream), and
            # aggregation runs two slots behind (its one-hot finished last
            # stream). TensorE then never waits mid-run.
            def emit_transp_assign(s):
                trT = transps.tile([P, nch_s, K], BF16, tag="tr", name="trT")
                for ch in range(nch_s):
                    nc.tensor.matmul(
                        trT[:, ch, :],
                        lhsT=s["simsSb"][:, ch * P:(ch + 1) * P],
                        rhs=ident_sb, is_transpose=True,
                        start=(ch == 0), stop=(ch == nch_s - 1),
                        skip_group_check=True,
                    )
                mx = mxp.tile([P, nch_s], F32, tag="mx", name="mx")
                nc.vector.tensor_reduce(
                    mx, trT, axis=mybir.AxisListType.X, op=Alu.max)
                asn = asnp.tile([P, nch_s, K], FP8, tag="asn", name="asn")
                nc.vector.scalar_tensor_tensor(
                    out=asn, in0=trT, scalar=1.0,
                    in1=mx[:, :, None].broadcast_to([P, nch_s, K]),
                    op0=Alu.mult, op1=Alu.is_ge,
                )
                s["asn"] = asn

            prev1 = None  # slot awaiting transpose+assign (1 behind)
            prev2 = None  # slot awaiting aggregation (2 behind)
            fin1_pending = None
            for t in range(nslots):
                b, sj = divmod(t, S)
                tsp, nat = tsps[t], nats[t]

                def emit_deferred():
                    nonlocal fin1_pending
                    if prev1 is not None:
                        emit_transp_assign(prev1)
                    if prev2 is not None:
                        emit_agg(prev2)
                        if prev2["sj"] == S - 1:
                            fin1_pending = emit_fin1(prev2["b"])

                def emit_fin2_pending():
                    nonlocal fin1_pending
                    if fin1_pending is not None and (
                            prev2 is None or prev2["sj"] == S - 1):
                        emit_fin2(fin1_pending)
                        fin1_pending = None

                # In the last streams the sims matmuls sit waiting on the
                # final DMAs; emit the (ready) deferred work ahead of them
                # so the drain isn't queued behind the DMA wait.
                # issues first: they have no data deps, so placing them at
                # the head of each engine's per-slot queue segment keeps the
                # rings fed even when the PE/ACT pipeline lags (otherwise a
                # late ACT copy gates the issue and the stream tail crawls)
                if t + 3 < nslots:
                    issue_tsp(t + 3, nc.sync)
                if t + 2 < nslots and t + 2 >= 2:
                    issue_nat(t + 2, nc.scalar)
                tail_stream = t >= nslots - 2
                if tail_stream:
                    pe_fill(5)
                    emit_deferred()
                    emit_fin2_pending()
                # sims^T: codebook-stationary DoubleRow accumulation
                simsT = simsps.tile([K, nsl], F32, tag="sims")
                for c in range(CP):
                    if t == 0 and c == CP // 2:
                        # slot 0 arrives as two DMA halves; bridge the PE
                        # through the wait for the second half so the HAM
                        # busy window isn't reset mid-warm-up
                        pe_fill(8)
                    nc.tensor.matmul(
                        simsT, lhsT=cnt_sb[:, c], rhs=tsp[:, c],
                        start=(c == 0), stop=(c == CP - 1),
                        perf_mode=DR, skip_group_check=True,
                    )
                simsSb = simsbp.tile([K, nsl], BF16, tag="simsb")
                nc.scalar.mul(simsSb, simsT, 1.0)
                if not tail_stream:
                    emit_deferred()
                    emit_fin2_pending()
                if t < 3:
                    # bridge the DMA-paced inter-slot gaps while the PE is
                    # still throttled: one un-interrupted ~3.4us busy window
                    # is needed to reach full clock, and any gap resets it
                    pe_fill(9)
                prev2 = prev1
                prev1 = dict(b=b, sj=sj, nat=nat, simsSb=simsSb)
            # drain: transpose/assign of the last slot, then the last two
            # slots' aggregation and the final image's finalize; PE fillers
            # between stages hold the clock gate at 8/8 through the
            # inter-engine dependency gaps
            pe_fill(5)
            emit_transp_assign(prev1)
            pe_fill(5)
            if prev2 is not None:
                emit_agg(prev2)
                if prev2["sj"] == S - 1:
                    emit_fin2(emit_fin1(prev2["b"]))
                pe_fill(5)
            emit_agg(prev1, last_img=True)
            if prev1["sj"] == S - 1:
                emit_fin_final(prev1)
            else:
                # single-slot images (shrunk sim builds): plain finalize
                emit_fin2(emit_fin1(prev1["b"]))

    _split_multi_waits(nc)
    return nc


def prep_inputs(query_descs, c_centers, imgs=IMGS, npair=NPAIR, ncores=NCORES):
    """Host-side layout prep shared by kernel() and tests."""
    S, pps, nsl = _slot_geom(npair)
    NN = npair * 2 * P
    qd = np.ascontiguousarray(query_descs, dtype=np.float32)
    cc = np.ascontiguousarray(c_centers, dtype=np.float32)
    # normalized descriptors at x64 scale (sweet spot for fp8e4m3); the
    # x64 factors cancel in argmax and under the downstream l2norms
    nrm = np.maximum(np.linalg.norm(qd, axis=-1, keepdims=True), EPS)
    dn8 = (qd / nrm * 64.0).astype(NP_FP8)  # [B', N', D]
    cn = cc / np.maximum(np.linalg.norm(cc, axis=1, keepdims=True), EPS)
    cnT64 = np.ascontiguousarray(cn.T * 64.0).astype(NP_FP8)  # [D, K]
    # row p = [c, q, k] flat so the device DMA is a contiguous 768B/partition
    cnt2 = np.ascontiguousarray(
        cnT64.reshape(CP, 2, P, K).transpose(2, 0, 1, 3)
    ).reshape(P, CP * 2 * K)
    cenid = np.concatenate(
        [cc.astype(NP_BF16), np.eye(K, dtype=NP_BF16)], axis=1
    )  # [K, D+K]
    in_maps = []
    for core in range(ncores):
        sh = dn8[core * imgs:(core + 1) * imgs, :NN]  # [imgs, NN, D]
        # nat row (b, s, p) = [cp, q, d] flat (6KB contiguous per packet)
        nat = np.ascontiguousarray(
            sh.reshape(imgs, S, pps, 2, P, D).transpose(0, 1, 4, 2, 3, 5)
        ).reshape(imgs * S * P, pps * 2 * D)
        # tsp row (b, s, p) = [c, q, n] flat with (c,q,n) = desc[b, n0+n, 256c+128q+p]
        shT = sh.transpose(0, 2, 1)  # [imgs, D, NN]
        tsp = np.ascontiguousarray(
            shT.reshape(imgs, CP, 2, P, S, nsl).transpose(0, 4, 3, 1, 2, 5)
        ).reshape(imgs * S * P, CP * 2 * nsl)
        in_maps.append({
            "descsn": nat,
            "descst": tsp,
            "cnt2": cnt2,
            "cenid": cenid,
        })
    return in_maps


_NC_CACHE = {}


def _get_nc():
    if "nc" not in _NC_CACHE:
        _NC_CACHE["nc"] = build_nc()
    return _NC_CACHE["nc"]


def kernel(query_descs, c_centers):
    in_maps = prep_inputs(query_descs, c_centers)
    nc = _get_nc()
    res = run_bass_kernel_spmd(nc, in_maps, core_ids=list(range(NCORES)))
    out = np.concatenate(
        [res.results[i]["out"] for i in range(NCORES)], axis=0
    )  # [B, K*D] bf16
    return out.astype(np.float32)
